# revision 40
# baseline (speedup 1.0000x reference)
"""AttenBlock (InstanceNorm + 1x1-conv QKV self-attention + residual) on 8 trn2 cores.

Problem (hardcoded): x [B=4, C=64, H=96, W=96] f32; wq/wk/wv/wo [64,64]; b* [64].
  h = instance_norm(x); q,k,v = conv1x1(h, w*, b*)
  o = softmax(q^T k / 8) @ v ; out = x + conv1x1(o, wo, bo)

Sharding: 8 cores = 4 samples x 2 query-halves (data parallel, no
collectives). Each core computes UNNORMALIZED attention for its 4608 query
rows and returns [65, 4608] = [projected-O^T ; softmax-denominator row];
the host divides, adds the residual, and exactly recomputes the rare rows
whose softmax overflowed fp8 range (detected via the denominator).

Device pipeline (all contractions ZERO-PADDED to 128 rows -- the PE
streams at half rate when the contraction is <= 64):
  1. bn_stats/bn_aggr -> mean/rstd. The instance norm is FOLDED INTO THE
     QKV WEIGHTS: W' = W diag(rstd) (ScalarE scale-copy) with bias row
     b' = -W' mean (tiny PE matvec) applied through the ones row the host
     ships inside xs/xq -- no normalize pass over the data.
  2. Q, K (bf16), V^T via PE; wo is folded into the V weight host-side
     (wv' = wo@wv), V^T lands as fp8e4m3 DoubleRow pairs VT[128,36,2,80]
     with a ones row for the denominator trick.
  3. Attention: flat pipeline over items (qblock, key-pair g), qblocks
     interleaved in pairs so consecutive P@V matmuls share VT[g]:
       - 2 fill matmuls (bf16, 128-contraction) -> S^T pair [128,2,512]
         in a 3-deep PSUM ring; row 64 of KK/QQ adds a constant D to
         every score (see below).
       - exp one item behind, p = e^(s/8 - 7) as fp8e5m2, alternating:
         ScalarE does exact exp (overflow -> +inf -> detected host-side);
         VectorE does a Schraudolph bit-trick, bits = min(s+D, M)*A8
         via a saturating uint8 convert (D rides the fills' row 64 so
         both ALU ops are free for the clamp; clamp -> den >= 57344 ->
         detected host-side). ~42.5M exps/core split across both engines
         at ~1 elem/cycle/lane each.
       - P@V two items behind as ONE fp8 DoubleRow matmul per item
         (256-key contraction) accumulating [65,512]; after the last
         accumulation the tile is copied out (ScalarE) and DMA'd.
Measured (repeat-loop differential): ~225us/core attention, ~258us full,
vs 474us baseline. PE-bound: per item ~2x250ns fills + ~190ns DR matmul;
exp ~620ns/item/engine rides just under the PE.
"""

import os

import numpy as np

import concourse.bass as bass
import concourse.mybir as mybir
import concourse.tile as tile
from concourse import bacc
from concourse.bass_utils import run_bass_kernel_spmd

F32 = mybir.dt.float32
F32R = mybir.dt.float32r
BF16 = mybir.dt.bfloat16
F8E4 = mybir.dt.float8e4
F8E5 = mybir.dt.float8e5
U8 = mybir.dt.uint8
I16 = mybir.dt.int16
AF = mybir.ActivationFunctionType
ALU = mybir.AluOpType
DR = mybir.MatmulPerfMode.DoubleRow

C = 64          # channels
CA = C + 1      # channels + ones row (denominator trick)
N = 9216        # H*W
NQ = 4608       # query rows per core
QB = 512        # q-block width
NQB = NQ // QB  # 9
NPR = 36        # key-chunk pairs (256 keys each)
VPAD = 80       # VT pair stride (>=2*CA? no: per-t stride, 16-aligned >= CA)
EPS = 1e-5
EXP_OFF = 7.0   # exp(s/8 - EXP_OFF): centers typical P inside fp8e5m2
                # range (overflowing rows are detected and host-repaired)

LN2 = float(np.log(2.0))
# constant added to every raw score via row 64 of the fills (KK row = 1,
# QQ row = D); bf16-exact, ~= B8/A8 so the VectorE Schraudolph's intercept
# needs no separate add op
DCONST = 26.875

_cache = {}


def _build(use_bias, repeat=1, p_mode=None, exp_split=None, repeat_all=False):
    """repeat>1 wraps the attention phase in a hardware loop (benchmarking
    only -- wall-clock deltas between repeat counts expose per-iteration
    device time despite ~1.5s of axon dispatch overhead)."""
    p_mode = p_mode or os.environ.get("ATT_P_MODE", "f8")
    exp_split = exp_split if exp_split is not None else (
        os.environ.get("ATT_EXP_SPLIT", "1") == "1")
    fp8 = p_mode == "f8"

    nc = bacc.Bacc()
    xs = nc.dram_tensor("xs", [128, N], F32R, kind="ExternalInput")
    xq = nc.dram_tensor("xq", [128, NQ], F32R, kind="ExternalInput")
    kpad = nc.dram_tensor("kpad", [64, N], BF16, kind="ExternalInput")
    qpad = nc.dram_tensor("qpad", [64, NQ], BF16, kind="ExternalInput")
    wqt = nc.dram_tensor("wqt", [C, C], F32, kind="ExternalInput")
    wkt = nc.dram_tensor("wkt", [C, C], F32, kind="ExternalInput")
    wvt = nc.dram_tensor("wvt", [C, C], F32, kind="ExternalInput")  # (wo@wv)^T
    bias_in = {}
    if use_bias:
        for nm in ("bq", "bk"):
            bias_in[nm] = nc.dram_tensor(nm, [C, 1], F32, kind="ExternalInput")
    out = nc.dram_tensor("out", [CA, NQ], F32, kind="ExternalOutput")

    import contextlib as _ctxlib
    with tile.TileContext(nc) as tc:
        with (
            tc.For_i(0, repeat, 1) if repeat > 1 and repeat_all
            else _ctxlib.nullcontext(),
            tc.tile_pool(name="persist", bufs=1) as persist,
            tc.tile_pool(name="attn_sb", bufs=8) as attn_sb,
            tc.tile_pool(name="outp_sb", bufs=4) as outp_sb,
            nc.allow_low_precision(reason="fp32r/bf16/fp8 attention"),
        ):
            # ---------------- phase 0: loads ----------------
            # xs/xq land directly as fp32r with a ones row at partition 64:
            # the instance norm is FOLDED INTO THE QKV WEIGHTS below
            # (W' = W diag(rstd), bias row b' = -W' mean via the ones row),
            # so no separate normalize pass over the data is needed.
            # everything is zero-padded to a 128-deep contraction: the PE
            # streams half-rate when the contraction dim is <= 64, so padded
            # 128-row matmuls are ~1.6x faster than 65-row ones
            xs65 = persist.tile([128, N], F32R)
            for d in range(8):
                eng = nc.sync if d % 2 == 0 else nc.gpsimd
                eng.dma_start(xs65[:, d * (N // 8):(d + 1) * (N // 8)],
                              xs[:, d * (N // 8):(d + 1) * (N // 8)])
            xq65 = persist.tile([128, NQ], F32R)
            for d in range(4):
                eng = nc.sync if d % 2 == 0 else nc.gpsimd
                eng.dma_start(xq65[:, d * (NQ // 4):(d + 1) * (NQ // 4)],
                              xq[:, d * (NQ // 4):(d + 1) * (NQ // 4)])
            wqt_sb = persist.tile([C, C], F32)
            nc.gpsimd.dma_start(wqt_sb[:], wqt[:])
            wkt_sb = persist.tile([C, C], F32)
            nc.gpsimd.dma_start(wkt_sb[:], wkt[:])
            wvt_sb = persist.tile([C, C], F32)
            nc.gpsimd.dma_start(wvt_sb[:], wvt[:])
            bias_sb = {}
            for nm, t in bias_in.items():
                bias_sb[nm] = persist.tile([1, C], F32, name=nm + "_sb")
                nc.sync.dma_start(bias_sb[nm][:], t[:])

            # exp bias for ScalarE: scores arrive pre-shifted by +D
            # (65th fill row), so bias = -(EXP_OFF + D/8)
            ebias = persist.tile([128, 1], F32)
            nc.vector.memset(ebias[:], -(EXP_OFF + DCONST / 8.0))

            # ---------------- phase 1: stats + folded weights ----------------
            with tc.tile_pool(name="stats", bufs=1) as stats_pool:
                stats = stats_pool.tile([C, N // 512, 6], F32)
                for j in range(N // 512):
                    nc.vector.bn_stats(
                        out=stats[:, j, :],
                        in_=xs65[0:C, j * 512:(j + 1) * 512])
                mv = stats_pool.tile([C, 2], F32)
                nc.vector.bn_aggr(out=mv[:], in_=stats[:])
                eps_t = stats_pool.tile([C, 1], F32)
                nc.vector.memset(eps_t[:], EPS)
                std = stats_pool.tile([C, 1], F32)
                nc.scalar.activation(std[:], mv[:, 1:2], AF.Sqrt, bias=eps_t[:])
                rstd = stats_pool.tile([C, 1], F32)
                nc.vector.reciprocal(rstd[:], std[:])
                # mr = -mean*rstd (fp32r, stationary operand of bias matvecs)
                mr = stats_pool.tile([C, 1], F32R)
                nc.vector.tensor_scalar(
                    out=mr[:], in0=mv[:, 0:1], scalar1=rstd[:], scalar2=-1.0,
                    op0=ALU.mult, op1=ALU.mult)

                # w65[0:64] = W^T diag->rows scaled by rstd; w65[64] = -W' mean
                w65 = {}
                with tc.tile_pool(name="wprep_ps", bufs=3,
                                  space="PSUM") as wprep_ps:
                    for nm, wsb, bname in (("q", wqt_sb, "bq"),
                                           ("k", wkt_sb, "bk"),
                                           ("v", wvt_sb, None)):
                        wt = persist.tile([128, C], F32R, name="w65" + nm)
                        w65[nm] = wt
                        # zero rows 64..127 via a scale-0 copy (memset of
                        # f32r fails the ISA value-type check); the bias-row
                        # DMA below then overwrites row 64
                        nc.scalar.activation(wt[C:128, :],
                                             xs65[C:128, 0:C],
                                             AF.Copy, scale=0.0)
                        nc.scalar.activation(wt[0:C, :], wsb[:], AF.Copy,
                                             scale=rstd[:])
                        pb = wprep_ps.tile([1, C], F32, tag="pb")
                        nc.tensor.matmul(pb[:], mr[:], wt[0:C, :],
                                         start=True, stop=True)
                        brow = persist.tile([1, C], F32R, name="brow" + nm)
                        if use_bias and bname is not None:
                            nc.vector.tensor_add(brow[:], pb[:],
                                                 bias_sb[bname][:])
                        else:
                            nc.vector.tensor_copy(brow[:], pb[:])
                        # cross-partition move into the weight's bias row
                        nc.sync.dma_start(wt[C:CA, :], brow[:])

                # ---------------- phase 3: Q, K, V^T ----------------
                # row 64 of KK/QQ adds the constant D to every raw score
                # (KK row = 1, QQ row = D): the VectorE Schraudolph then
                # spends its two ALU ops on min(s'', MCLAMP) * A8 -- a
                # saturating clamp at fp8e5m2's top -- with the intercept
                # B8 = A8*D built in. Rows 65..127 are zero padding.
                QQ = persist.tile([128, NQ], BF16)
                nc.scalar.dma_start(QQ[C:128, :], qpad[:])
                KK = persist.tile([128, N], BF16)
                nc.scalar.dma_start(KK[C:128, :], kpad[:])
                vt_dt = F8E4 if fp8 else BF16
                VT = persist.tile([128, NPR, 2, VPAD], vt_dt)
                nc.gpsimd.memset(VT[:, :, :, C:CA], 1.0)


                with tc.tile_pool(name="qkv_ps", bufs=2, space="PSUM") as qkv_ps:
                    for j in range(NQB):  # Q first: copies fill DVE's idle
                        sl = slice(j * QB, (j + 1) * QB)
                        pq = qkv_ps.tile([C, QB], F32, tag="pq")
                        nc.tensor.matmul(pq[:], w65["q"][:], xq65[:, sl],
                                         start=True, stop=True)
                        nc.vector.tensor_copy(QQ[0:C, sl], pq[:])
                    for j in range(N // QB):  # K over all 9216 cols
                        sl = slice(j * QB, (j + 1) * QB)
                        pk = qkv_ps.tile([C, QB], F32, tag="pk")
                        nc.tensor.matmul(pk[:], w65["k"][:], xs65[:, sl],
                                         start=True, stop=True)
                        nc.scalar.copy(KK[0:C, sl], pk[:])
                    for g8 in range(9):  # V^T chunks [n, c], 8 per group
                        pv = qkv_ps.tile([128, 8, C], F32, tag="pv")
                        for u in range(8):
                            nb = g8 * 8 + u
                            nc.tensor.matmul(
                                pv[:, u, :],
                                xs65[:, nb * 128:(nb + 1) * 128],
                                w65["v"][:],
                                start=(u == 0), stop=(u == 7),
                            )
                        dst = VT[:, g8 * 4:(g8 + 1) * 4, :, 0:C]
                        src = pv[:].rearrange("p (a b) c -> p a b c", a=4)
                        nc.scalar.copy(dst, src)

            # ---------------- phase 4: attention ----------------
            import contextlib

            # Both engines produce p = e^(s/8 - EXP_OFF) in fp8e5m2:
            #  - ScalarE: exact exp + RNE convert. Overflow -> +inf -> that
            #    query's denominator is +inf -> host detects & repairs.
            #  - VectorE: Schraudolph bits = min(s'', MCLAMP)*A8 with a
            #    saturating uint8 convert (clamped [0, 123] - never invalid;
            #    clamped rows have den >= 57344 -> host detects & repairs).
            A8 = float(4.0 * 0.125 / LN2)
            MCLAMP = float(123.0 / A8)

            with (
                tc.tile_pool(name="st_ps", bufs=3, space="PSUM") as st_ps,
                tc.tile_pool(name="po_ps", bufs=2, space="PSUM") as po_ps,
                tc.For_i(0, repeat, 1) if repeat > 1 and not repeat_all
                else contextlib.nullcontext(),
            ):
                # work items: (qblock, key-pair g); qblocks of a pair
                # interleave so consecutive P@V matmuls share VT[g]
                items = []
                qpairs = [(0, 1), (2, 3), (4, 5), (6, 7), (8,)]
                for qp in qpairs:
                    for g in range(NPR):
                        for q in qp:
                            items.append((q, g))

                T = len(items)
                sts = {}
                pts = {}
                pos = {}

                def qsl(qb):
                    return slice(qb * QB, (qb + 1) * QB)

                def emit_fills(t):
                    q, g = items[t]
                    st = st_ps.tile([128, 2, QB], F32, tag="st")
                    sts[t] = st
                    for c in (2 * g, 2 * g + 1):
                        nc.tensor.matmul(
                            st[:, c % 2, :], KK[:, c * 128:(c + 1) * 128],
                            QQ[:, qsl(q)], start=True, stop=True)

                A16 = float(128.0 * 0.125 / LN2)
                B16 = float(127 * 128 - 0.043 * 128 - 128.0 * EXP_OFF / LN2
                            - A16 * DCONST)

                def emit_exp(t):
                    q, g = items[t]
                    st = sts.pop(t)
                    anum = int(os.environ.get("ATT_ACT_NUM", "1"))
                    aden = int(os.environ.get("ATT_ACT_DEN", "2"))
                    use_act = ((t + 1) * anum // aden > t * anum // aden
                               or not exp_split)
                    if fp8:
                        pt = attn_sb.tile([128, 2, QB], U8, tag="pt8")
                    else:
                        pt = attn_sb.tile([128, 2, QB], I16, tag="ptb")
                    src = st[:].rearrange("p a b -> p (a b)")
                    dst = pt[:].rearrange("p a b -> p (a b)")
                    if use_act:
                        adst = dst.bitcast(F8E5 if fp8 else BF16)
                        nc.scalar.activation(adst, src, AF.Exp,
                                             scale=0.125, bias=ebias[:])
                    elif fp8:
                        nc.vector.tensor_scalar(
                            out=dst, in0=src, scalar1=MCLAMP, scalar2=A8,
                            op0=ALU.min, op1=ALU.mult)
                    else:
                        nc.vector.tensor_scalar(
                            out=dst, in0=src, scalar1=A16, scalar2=B16,
                            op0=ALU.mult, op1=ALU.add)
                    pts[t] = pt

                def emit_pv(t):
                    q, g = items[t]
                    if g == 0:
                        pos[q] = po_ps.tile([CA, QB], F32, tag="po",
                                            name="po")
                    po = pos[q]
                    pt = pts.pop(t)
                    if fp8:
                        nc.tensor.matmul(
                            po[:], VT[:, g, :, 0:CA], pt[:].bitcast(F8E5),
                            start=(g == 0), stop=(g == NPR - 1),
                            perf_mode=DR)
                    else:
                        for c in (0, 1):
                            nc.tensor.matmul(
                                po[:], VT[:, g, c, 0:CA],
                                pt[:, c, :].bitcast(BF16),
                                start=(g == 0 and c == 0),
                                stop=(g == NPR - 1 and c == 1))
                    if g == NPR - 1:
                        ot = outp_sb.tile([CA, QB], F32, tag="ot")
                        nc.scalar.copy(ot[:], pos.pop(q)[:])
                        nc.sync.dma_start(out[:, qsl(q)], ot[:])

                PV_LAG = int(os.environ.get("ATT_PV_LAG", "3"))
                for t in range(T + PV_LAG):
                    if t < T:
                        emit_fills(t)
                    if 1 <= t < T + 1:
                        emit_exp(t - 1)
                    if t >= PV_LAG:
                        emit_pv(t - PV_LAG)

    nc.compile()
    return nc


def _get_nc(use_bias):
    key = ("nc", use_bias)
    if key not in _cache:
        _cache[key] = _build(use_bias)
    return _cache[key]


def _make_in_maps(x, wq, bq, wk, bk, wv, bv, wo, bo, use_bias):
    import ml_dtypes
    ws = {
        "wqt": np.ascontiguousarray(wq.T.astype(np.float32)),
        "wkt": np.ascontiguousarray(wk.T.astype(np.float32)),
        "wvt": np.ascontiguousarray(
            (wo.astype(np.float64) @ wv.astype(np.float64)).T.astype(np.float32)),
    }
    if use_bias:
        for nm, b in (("bq", bq), ("bk", bk)):
            ws[nm] = np.ascontiguousarray(b.astype(np.float32).reshape(1, C))
    in_maps = []
    ones_n = np.ones((1, N), dtype=np.float32)
    zeros_n = np.zeros((128 - CA, N), dtype=np.float32)
    # KK/QQ pad rows: row 64 carries the fills' score-offset constant
    # (KK row = 1.0, QQ row = DCONST), rows 65..127 are zeros
    kpad = np.zeros((64, N), dtype=ml_dtypes.bfloat16)
    kpad[0, :] = 1.0
    qpad = np.zeros((64, NQ), dtype=ml_dtypes.bfloat16)
    qpad[0, :] = DCONST
    for core in range(8):
        b, half = core // 2, core % 2
        xsf = np.ascontiguousarray(np.concatenate(
            [x[b].reshape(C, N).astype(np.float32), ones_n, zeros_n],
            axis=0))
        xqf = np.ascontiguousarray(xsf[:, half * NQ:(half + 1) * NQ])
        in_maps.append({"xs": xsf, "xq": xqf, "kpad": kpad, "qpad": qpad,
                        **ws})
    return in_maps


def run(inputs, trace=False):
    inputs = {k: np.asarray(v) for k, v in inputs.items()}
    use_bias = any(
        np.any(inputs[nm]) for nm in ("bq", "bk", "bv", "bo")
    )
    nc = _get_nc(use_bias)
    in_maps = _make_in_maps(use_bias=use_bias, **inputs)
    res = run_bass_kernel_spmd(nc, in_maps, list(range(8)), trace=trace)
    x = inputs["x"]
    B = x.shape[0]
    H = W = 96
    # host-side unshard: out = x + O/den (+ wo@bv + bo)
    if use_bias:
        bsum = (inputs["wo"].astype(np.float64) @ inputs["bv"].astype(np.float64)
                + inputs["bo"].astype(np.float64))
    else:
        bsum = np.zeros((C,), dtype=np.float64)
    full = np.empty((B, C, H, W), dtype=np.float32)
    xr = x.reshape(B, C, N).astype(np.float64)
    kv_cache = {}

    def batch_kv(b):
        # exact K/V/h for repair rows (rows whose softmax hit the fp8
        # clamp, detected via denominator >= fp8e5m2 max)
        if b not in kv_cache:
            xb = xr[b]
            mean = xb.mean(1, keepdims=True)
            var = xb.var(1, keepdims=True)
            h = (xb - mean) / np.sqrt(var + EPS)
            kv = inputs["wk"].astype(np.float64) @ h                 + inputs["bk"].astype(np.float64)[:, None]
            vt = (inputs["wo"].astype(np.float64)
                  @ inputs["wv"].astype(np.float64)) @ h                 + (inputs["wo"].astype(np.float64)
                   @ inputs["bv"].astype(np.float64))[:, None]
            qv = inputs["wq"].astype(np.float64) @ h                 + inputs["bq"].astype(np.float64)[:, None]
            kv_cache[b] = (qv, kv, vt)
        return kv_cache[b]

    for core in range(8):
        b, half = core // 2, core % 2
        oden = res.results[core]["out"].astype(np.float64)  # [CA, NQ]
        den = oden[C]
        bad = ~(den < 5.0e4)  # catches clamp-saturated rows, inf and nan
        o = oden[0:C] / np.where(bad, 1.0, den)
        if bad.any():
            qv, kv, vt = batch_kv(b)
            idx = np.nonzero(bad)[0]
            qn = qv[:, half * NQ + idx]              # [C, nbad]
            srow = (qn.T @ kv) * 0.125               # [nbad, N]
            srow -= srow.max(1, keepdims=True)
            p = np.exp(srow)
            o[:, idx] = (vt @ p.T) / p.sum(1)
        sl = slice(half * NQ, (half + 1) * NQ)
        full.reshape(B, C, N)[b, :, sl.start:sl.stop] = (
            xr[b][:, sl] + o + bsum[:, None]).astype(np.float32)
    return full, res


def kernel(**inputs):
    return run(inputs, trace=False)[0]


# revision 41
# speedup vs baseline: 1.1784x; 1.1784x over previous
"""AttenBlock (InstanceNorm + 1x1-conv QKV self-attention + residual) on 8 trn2 cores.

Problem (hardcoded): x [B=4, C=64, H=96, W=96] f32; wq/wk/wv/wo [64,64]; b* [64].
  h = instance_norm(x); q,k,v = conv1x1(h, w*, b*)
  o = softmax(q^T k / 8) @ v ; out = x + conv1x1(o, wo, bo)

Sharding: 8 cores = 4 samples x 2 query-halves (data parallel, no
collectives). Each core computes UNNORMALIZED attention for its 4608 query
rows and returns [65, 4608] = [projected-O^T ; softmax-denominator row];
the host divides, adds the residual, and exactly recomputes the rare rows
whose softmax overflowed fp8 range (detected via the denominator).

Device pipeline (all contractions ZERO-PADDED to 128 rows -- the PE
streams at half rate when the contraction is <= 64):
  1. bn_stats/bn_aggr -> mean/rstd. The instance norm is FOLDED INTO THE
     QKV WEIGHTS: W' = W diag(rstd) (ScalarE scale-copy) with bias row
     b' = -W' mean (tiny PE matvec) applied through the ones row the host
     ships inside xs/xq -- no normalize pass over the data.
  2. Q, K (bf16), V^T via PE; wo is folded into the V weight host-side
     (wv' = wo@wv), V^T lands as fp8e4m3 DoubleRow pairs VT[128,36,2,80]
     with a ones row for the denominator trick.
  3. Attention: flat pipeline over items (qblock, key-pair g), qblocks
     interleaved in pairs so consecutive P@V matmuls share VT[g]:
       - 2 fill matmuls (bf16, 128-contraction) -> S^T pair [128,2,512]
         in a 3-deep PSUM ring; row 64 of KK/QQ adds a constant D to
         every score (see below).
       - exp one item behind, p = e^(s/8 - 7) as fp8e5m2, alternating:
         ScalarE does exact exp (overflow -> +inf -> detected host-side);
         VectorE does a Schraudolph bit-trick, bits = min(s+D, M)*A8
         via a saturating uint8 convert (D rides the fills' row 64 so
         both ALU ops are free for the clamp; clamp -> den >= 57344 ->
         detected host-side). ~42.5M exps/core split across both engines
         at ~1 elem/cycle/lane each.
       - P@V two items behind as ONE fp8 DoubleRow matmul per item
         (256-key contraction) accumulating [65,512]; after the last
         accumulation the tile is copied out (ScalarE) and DMA'd.
Measured (repeat-loop differential): ~225us/core attention, ~258us full,
vs 474us baseline. PE-bound: per item ~2x250ns fills + ~190ns DR matmul;
exp ~620ns/item/engine rides just under the PE.
"""

import os

import numpy as np

import concourse.bass as bass
import concourse.mybir as mybir
import concourse.tile as tile
from concourse import bacc
from concourse.bass_utils import run_bass_kernel_spmd

F32 = mybir.dt.float32
F32R = mybir.dt.float32r
BF16 = mybir.dt.bfloat16
F8E4 = mybir.dt.float8e4
F8E5 = mybir.dt.float8e5
U8 = mybir.dt.uint8
I16 = mybir.dt.int16
AF = mybir.ActivationFunctionType
ALU = mybir.AluOpType
DR = mybir.MatmulPerfMode.DoubleRow

C = 64          # channels
CA = C + 1      # channels + ones row (denominator trick)
N = 9216        # H*W
NQ = 4608       # query rows per core
QB = 512        # q-block width
NQB = NQ // QB  # 9
NPR = 36        # key-chunk pairs (256 keys each)
VPAD = 80       # VT pair stride (>=2*CA? no: per-t stride, 16-aligned >= CA)
EPS = 1e-5
EXP_OFF = 7.0   # exp(s/8 - EXP_OFF): centers typical P inside fp8e5m2
                # range (overflowing rows are detected and host-repaired)

LN2 = float(np.log(2.0))
# constant added to every raw score via row 64 of the fills (KK row = 1,
# QQ row = D); bf16-exact, ~= B8/A8 so the VectorE Schraudolph's intercept
# needs no separate add op
DCONST = 26.875

_cache = {}


def _build(use_bias, repeat=1, p_mode=None, exp_split=None, repeat_all=False):
    """repeat>1 wraps the attention phase in a hardware loop (benchmarking
    only -- wall-clock deltas between repeat counts expose per-iteration
    device time despite ~1.5s of axon dispatch overhead)."""
    p_mode = p_mode or os.environ.get("ATT_P_MODE", "f8")
    exp_split = exp_split if exp_split is not None else (
        os.environ.get("ATT_EXP_SPLIT", "1") == "1")
    fp8 = p_mode == "f8"

    nc = bacc.Bacc()
    xs = nc.dram_tensor("xs", [128, N], F32R, kind="ExternalInput")
    xq = nc.dram_tensor("xq", [128, NQ], F32R, kind="ExternalInput")
    kpad = nc.dram_tensor("kpad", [64, N], BF16, kind="ExternalInput")
    qpad = nc.dram_tensor("qpad", [64, NQ], BF16, kind="ExternalInput")
    wqt = nc.dram_tensor("wqt", [C, C], F32, kind="ExternalInput")
    wkt = nc.dram_tensor("wkt", [C, C], F32, kind="ExternalInput")
    wvt = nc.dram_tensor("wvt", [C, C], F32, kind="ExternalInput")  # (wo@wv)^T
    bias_in = {}
    if use_bias:
        for nm in ("bq", "bk"):
            bias_in[nm] = nc.dram_tensor(nm, [C, 1], F32, kind="ExternalInput")
    out = nc.dram_tensor("out", [CA, NQ], F32, kind="ExternalOutput")

    import contextlib as _ctxlib
    with tile.TileContext(nc) as tc:
        with (
            tc.For_i(0, repeat, 1) if repeat > 1 and repeat_all
            else _ctxlib.nullcontext(),
            tc.tile_pool(name="persist", bufs=1) as persist,
            tc.tile_pool(name="attn_sb", bufs=6) as attn_sb,
            tc.tile_pool(name="outp_sb", bufs=4) as outp_sb,
            nc.allow_low_precision(reason="fp32r/bf16/fp8 attention"),
        ):
            # ---------------- phase 0: loads ----------------
            # xs/xq land directly as fp32r with a ones row at partition 64:
            # the instance norm is FOLDED INTO THE QKV WEIGHTS below
            # (W' = W diag(rstd), bias row b' = -W' mean via the ones row),
            # so no separate normalize pass over the data is needed.
            # everything is zero-padded to a 128-deep contraction: the PE
            # streams half-rate when the contraction dim is <= 64, so padded
            # 128-row matmuls are ~1.6x faster than 65-row ones
            xs65 = persist.tile([128, N], F32R)
            for d in range(4):
                eng = nc.sync if d % 2 == 0 else nc.gpsimd
                eng.dma_start(xs65[:, d * (N // 4):(d + 1) * (N // 4)],
                              xs[:, d * (N // 4):(d + 1) * (N // 4)])
            xq65 = persist.tile([128, NQ], F32R)
            for d in range(2):
                nc.sync.dma_start(xq65[:, d * (NQ // 2):(d + 1) * (NQ // 2)],
                                  xq[:, d * (NQ // 2):(d + 1) * (NQ // 2)])
            wqt_sb = persist.tile([C, C], F32)
            nc.gpsimd.dma_start(wqt_sb[:], wqt[:])
            wkt_sb = persist.tile([C, C], F32)
            nc.gpsimd.dma_start(wkt_sb[:], wkt[:])
            wvt_sb = persist.tile([C, C], F32)
            nc.gpsimd.dma_start(wvt_sb[:], wvt[:])
            bias_sb = {}
            for nm, t in bias_in.items():
                bias_sb[nm] = persist.tile([1, C], F32, name=nm + "_sb")
                nc.sync.dma_start(bias_sb[nm][:], t[:])

            # exp bias for ScalarE: scores arrive pre-shifted by +D
            # (65th fill row), so bias = -(EXP_OFF + D/8)
            ebias = persist.tile([128, 1], F32)
            nc.vector.memset(ebias[:], -(EXP_OFF + DCONST / 8.0))

            # ---------------- phase 1: stats + folded weights ----------------
            with tc.tile_pool(name="stats", bufs=1) as stats_pool:
                stats = stats_pool.tile([C, N // 512, 6], F32)
                for j in range(N // 512):
                    nc.vector.bn_stats(
                        out=stats[:, j, :],
                        in_=xs65[0:C, j * 512:(j + 1) * 512])
                mv = stats_pool.tile([C, 2], F32)
                nc.vector.bn_aggr(out=mv[:], in_=stats[:])
                eps_t = stats_pool.tile([C, 1], F32)
                nc.vector.memset(eps_t[:], EPS)
                std = stats_pool.tile([C, 1], F32)
                nc.scalar.activation(std[:], mv[:, 1:2], AF.Sqrt, bias=eps_t[:])
                rstd = stats_pool.tile([C, 1], F32)
                nc.vector.reciprocal(rstd[:], std[:])
                # mr = -mean*rstd (fp32r, stationary operand of bias matvecs)
                mr = stats_pool.tile([C, 1], F32R)
                nc.vector.tensor_scalar(
                    out=mr[:], in0=mv[:, 0:1], scalar1=rstd[:], scalar2=-1.0,
                    op0=ALU.mult, op1=ALU.mult)

                # w65[0:64] = W^T diag->rows scaled by rstd; w65[64] = -W' mean
                w65 = {}
                with tc.tile_pool(name="wprep_ps", bufs=3,
                                  space="PSUM") as wprep_ps:
                    for nm, wsb, bname in (("q", wqt_sb, "bq"),
                                           ("k", wkt_sb, "bk"),
                                           ("v", wvt_sb, None)):
                        wt = persist.tile([128, C], F32R, name="w65" + nm)
                        w65[nm] = wt
                        # zero rows 64..127 via a scale-0 copy (memset of
                        # f32r fails the ISA value-type check); the bias-row
                        # DMA below then overwrites row 64
                        nc.scalar.activation(wt[C:128, :],
                                             xs65[C:128, 0:C],
                                             AF.Copy, scale=0.0)
                        nc.scalar.activation(wt[0:C, :], wsb[:], AF.Copy,
                                             scale=rstd[:])
                        pb = wprep_ps.tile([1, C], F32, tag="pb")
                        nc.tensor.matmul(pb[:], mr[:], wt[0:C, :],
                                         start=True, stop=True)
                        brow = persist.tile([1, C], F32R, name="brow" + nm)
                        if use_bias and bname is not None:
                            nc.vector.tensor_add(brow[:], pb[:],
                                                 bias_sb[bname][:])
                        else:
                            nc.vector.tensor_copy(brow[:], pb[:])
                        # cross-partition move into the weight's bias row
                        nc.sync.dma_start(wt[C:CA, :], brow[:])

                # ---------------- phase 3: Q, K, V^T ----------------
                # row 64 of KK/QQ adds the constant D to every raw score
                # (KK row = 1, QQ row = D): the VectorE Schraudolph then
                # spends its two ALU ops on min(s'', MCLAMP) * A8 -- a
                # saturating clamp at fp8e5m2's top -- with the intercept
                # B8 = A8*D built in. Rows 65..127 are zero padding.
                QQ = persist.tile([128, NQ], BF16)
                nc.scalar.dma_start(QQ[C:128, :], qpad[:])
                KK = persist.tile([128, N], BF16)
                nc.scalar.dma_start(KK[C:128, :], kpad[:])
                vt_dt = F8E4 if fp8 else BF16
                VT = persist.tile([128, NPR, 2, VPAD], vt_dt)
                nc.gpsimd.memset(VT[:, :, :, C:CA], 1.0)


                with tc.tile_pool(name="qkv_ps", bufs=2, space="PSUM") as qkv_ps:
                    for j in range(NQB):  # Q first: copies fill DVE's idle
                        sl = slice(j * QB, (j + 1) * QB)
                        pq = qkv_ps.tile([C, QB], F32, tag="pq")
                        nc.tensor.matmul(pq[:], w65["q"][:], xq65[:, sl],
                                         start=True, stop=True)
                        nc.vector.tensor_copy(QQ[0:C, sl], pq[:])
                    for j in range(N // QB):  # K over all 9216 cols
                        sl = slice(j * QB, (j + 1) * QB)
                        pk = qkv_ps.tile([C, QB], F32, tag="pk")
                        nc.tensor.matmul(pk[:], w65["k"][:], xs65[:, sl],
                                         start=True, stop=True)
                        nc.scalar.copy(KK[0:C, sl], pk[:])
                    for g8 in range(9):  # V^T chunks [n, c], 8 per group
                        pv = qkv_ps.tile([128, 8, C], F32, tag="pv")
                        for u in range(8):
                            nb = g8 * 8 + u
                            nc.tensor.matmul(
                                pv[:, u, :],
                                xs65[:, nb * 128:(nb + 1) * 128],
                                w65["v"][:],
                                start=(u == 0), stop=(u == 7),
                            )
                        dst = VT[:, g8 * 4:(g8 + 1) * 4, :, 0:C]
                        src = pv[:].rearrange("p (a b) c -> p a b c", a=4)
                        nc.scalar.copy(dst, src)

            # ---------------- phase 4: attention ----------------
            import contextlib

            # Both engines produce p = e^(s/8 - EXP_OFF) in fp8e5m2:
            #  - ScalarE: exact exp + RNE convert. Overflow -> +inf -> that
            #    query's denominator is +inf -> host detects & repairs.
            #  - VectorE: Schraudolph bits = min(s'', MCLAMP)*A8 with a
            #    saturating uint8 convert (clamped [0, 123] - never invalid;
            #    clamped rows have den >= 57344 -> host detects & repairs).
            A8 = float(4.0 * 0.125 / LN2)
            MCLAMP = float(123.0 / A8)

            with (
                tc.tile_pool(name="st_ps", bufs=3, space="PSUM") as st_ps,
                tc.tile_pool(name="po_ps", bufs=2, space="PSUM") as po_ps,
                tc.For_i(0, repeat, 1) if repeat > 1 and not repeat_all
                else contextlib.nullcontext(),
            ):
                # work items: (qblock, key-pair g); qblocks of a pair
                # interleave so consecutive P@V matmuls share VT[g]
                items = []
                qpairs = [(0, 1), (2, 3), (4, 5), (6, 7), (8,)]
                for qp in qpairs:
                    for g in range(NPR):
                        for q in qp:
                            items.append((q, g))

                T = len(items)
                sts = {}
                pts = {}
                pos = {}

                def qsl(qb):
                    return slice(qb * QB, (qb + 1) * QB)

                def emit_fills(t):
                    q, g = items[t]
                    st = st_ps.tile([128, 2, QB], F32, tag="st")
                    sts[t] = st
                    for c in (2 * g, 2 * g + 1):
                        nc.tensor.matmul(
                            st[:, c % 2, :], KK[:, c * 128:(c + 1) * 128],
                            QQ[:, qsl(q)], start=True, stop=True)

                A16 = float(128.0 * 0.125 / LN2)
                B16 = float(127 * 128 - 0.043 * 128 - 128.0 * EXP_OFF / LN2
                            - A16 * DCONST)

                def emit_exp(t):
                    q, g = items[t]
                    st = sts.pop(t)
                    anum = int(os.environ.get("ATT_ACT_NUM", "1"))
                    aden = int(os.environ.get("ATT_ACT_DEN", "2"))
                    use_act = ((t + 1) * anum // aden > t * anum // aden
                               or not exp_split)
                    if fp8:
                        pt = attn_sb.tile([128, 2, QB], U8, tag="pt8")
                    else:
                        pt = attn_sb.tile([128, 2, QB], I16, tag="ptb")
                    src = st[:].rearrange("p a b -> p (a b)")
                    dst = pt[:].rearrange("p a b -> p (a b)")
                    if use_act:
                        adst = dst.bitcast(F8E5 if fp8 else BF16)
                        nc.scalar.activation(adst, src, AF.Exp,
                                             scale=0.125, bias=ebias[:])
                    elif fp8:
                        nc.vector.tensor_scalar(
                            out=dst, in0=src, scalar1=MCLAMP, scalar2=A8,
                            op0=ALU.min, op1=ALU.mult)
                    else:
                        nc.vector.tensor_scalar(
                            out=dst, in0=src, scalar1=A16, scalar2=B16,
                            op0=ALU.mult, op1=ALU.add)
                    pts[t] = pt

                def emit_pv(t):
                    q, g = items[t]
                    if g == 0:
                        pos[q] = po_ps.tile([CA, QB], F32, tag="po",
                                            name="po")
                    po = pos[q]
                    pt = pts.pop(t)
                    if fp8:
                        nc.tensor.matmul(
                            po[:], VT[:, g, :, 0:CA], pt[:].bitcast(F8E5),
                            start=(g == 0), stop=(g == NPR - 1),
                            perf_mode=DR)
                    else:
                        for c in (0, 1):
                            nc.tensor.matmul(
                                po[:], VT[:, g, c, 0:CA],
                                pt[:, c, :].bitcast(BF16),
                                start=(g == 0 and c == 0),
                                stop=(g == NPR - 1 and c == 1))
                    if g == NPR - 1:
                        ot = outp_sb.tile([CA, QB], F32, tag="ot")
                        nc.scalar.copy(ot[:], pos.pop(q)[:])
                        nc.sync.dma_start(out[:, qsl(q)], ot[:])

                PV_LAG = int(os.environ.get("ATT_PV_LAG", "3"))
                for t in range(T + PV_LAG):
                    if t < T:
                        emit_fills(t)
                    if 1 <= t < T + 1:
                        emit_exp(t - 1)
                    if t >= PV_LAG:
                        emit_pv(t - PV_LAG)

    nc.compile()
    return nc


def _get_nc(use_bias):
    key = ("nc", use_bias)
    if key not in _cache:
        _cache[key] = _build(use_bias)
    return _cache[key]


def _make_in_maps(x, wq, bq, wk, bk, wv, bv, wo, bo, use_bias):
    import ml_dtypes
    ws = {
        "wqt": np.ascontiguousarray(wq.T.astype(np.float32)),
        "wkt": np.ascontiguousarray(wk.T.astype(np.float32)),
        "wvt": np.ascontiguousarray(
            (wo.astype(np.float64) @ wv.astype(np.float64)).T.astype(np.float32)),
    }
    if use_bias:
        for nm, b in (("bq", bq), ("bk", bk)):
            ws[nm] = np.ascontiguousarray(b.astype(np.float32).reshape(1, C))
    in_maps = []
    ones_n = np.ones((1, N), dtype=np.float32)
    zeros_n = np.zeros((128 - CA, N), dtype=np.float32)
    # KK/QQ pad rows: row 64 carries the fills' score-offset constant
    # (KK row = 1.0, QQ row = DCONST), rows 65..127 are zeros
    kpad = np.zeros((64, N), dtype=ml_dtypes.bfloat16)
    kpad[0, :] = 1.0
    qpad = np.zeros((64, NQ), dtype=ml_dtypes.bfloat16)
    qpad[0, :] = DCONST
    for core in range(8):
        b, half = core // 2, core % 2
        xsf = np.ascontiguousarray(np.concatenate(
            [x[b].reshape(C, N).astype(np.float32), ones_n, zeros_n],
            axis=0))
        xqf = np.ascontiguousarray(xsf[:, half * NQ:(half + 1) * NQ])
        in_maps.append({"xs": xsf, "xq": xqf, "kpad": kpad, "qpad": qpad,
                        **ws})
    return in_maps


def run(inputs, trace=False):
    inputs = {k: np.asarray(v) for k, v in inputs.items()}
    use_bias = any(
        np.any(inputs[nm]) for nm in ("bq", "bk", "bv", "bo")
    )
    nc = _get_nc(use_bias)
    in_maps = _make_in_maps(use_bias=use_bias, **inputs)
    res = run_bass_kernel_spmd(nc, in_maps, list(range(8)), trace=trace)
    x = inputs["x"]
    B = x.shape[0]
    H = W = 96
    # host-side unshard: out = x + O/den (+ wo@bv + bo)
    if use_bias:
        bsum = (inputs["wo"].astype(np.float64) @ inputs["bv"].astype(np.float64)
                + inputs["bo"].astype(np.float64))
    else:
        bsum = np.zeros((C,), dtype=np.float64)
    full = np.empty((B, C, H, W), dtype=np.float32)
    xr = x.reshape(B, C, N).astype(np.float64)
    kv_cache = {}

    def batch_kv(b):
        # exact K/V/h for repair rows (rows whose softmax hit the fp8
        # clamp, detected via denominator >= fp8e5m2 max)
        if b not in kv_cache:
            xb = xr[b]
            mean = xb.mean(1, keepdims=True)
            var = xb.var(1, keepdims=True)
            h = (xb - mean) / np.sqrt(var + EPS)
            kv = inputs["wk"].astype(np.float64) @ h                 + inputs["bk"].astype(np.float64)[:, None]
            vt = (inputs["wo"].astype(np.float64)
                  @ inputs["wv"].astype(np.float64)) @ h                 + (inputs["wo"].astype(np.float64)
                   @ inputs["bv"].astype(np.float64))[:, None]
            qv = inputs["wq"].astype(np.float64) @ h                 + inputs["bq"].astype(np.float64)[:, None]
            kv_cache[b] = (qv, kv, vt)
        return kv_cache[b]

    for core in range(8):
        b, half = core // 2, core % 2
        oden = res.results[core]["out"].astype(np.float64)  # [CA, NQ]
        den = oden[C]
        bad = ~(den < 5.0e4)  # catches clamp-saturated rows, inf and nan
        o = oden[0:C] / np.where(bad, 1.0, den)
        if bad.any():
            qv, kv, vt = batch_kv(b)
            idx = np.nonzero(bad)[0]
            qn = qv[:, half * NQ + idx]              # [C, nbad]
            srow = (qn.T @ kv) * 0.125               # [nbad, N]
            srow -= srow.max(1, keepdims=True)
            p = np.exp(srow)
            o[:, idx] = (vt @ p.T) / p.sum(1)
        sl = slice(half * NQ, (half + 1) * NQ)
        full.reshape(B, C, N)[b, :, sl.start:sl.stop] = (
            xr[b][:, sl] + o + bsum[:, None]).astype(np.float32)
    return full, res


def kernel(**inputs):
    return run(inputs, trace=False)[0]
